# revision 6
# baseline (speedup 1.0000x reference)
"""GCN encoder (GCNConv + relu + global_add_pool) as a distributed Bass kernel
for 8 TRN2 NeuronCores. v2: chunk-major main loop with pipelined chunked
AllGathers, one big gather call per (chunk, supertile), ReduceScatter+AllGather
tail.

Sharding strategy (edge-parallel per the hint):
  - Nodes sharded contiguously: core k owns nodes [k*NSH, (k+1)*NSH).
  - Edges partitioned by TARGET owner; each core aggregates its own targets.
  - Per-core z shard z_k = dinv_k * (x_k @ W) built on device. The global
    z table is split into 4 chunks (quarters of every core's shard,
    interleaved core-major) and each chunk AllGathered separately so the
    main loop can start after chunk 0 lands.
  - Per-edge messages fetched with dma_gather (256B rows), aggregated per
    64-target window with one-hot matmuls on the TensorEngine (PSUM accum),
    chunk partials accumulated in SBUF.
  - Self loops folded in analytically: agg[t] = dinv[t] * (S[t] + z[t]).
  - relu, then graph pooling via one-hot matmul, ReduceScatter + AllGather.

Host-side prep does ONLY index/layout work (bucketing, sorting, one-hot
encodings of index structure, CSR-style cumulative counts). All value math
(degree arithmetic, rsqrt, x@W, scaling, aggregation, relu, pooling) runs on
device.
"""

import numpy as np
import ml_dtypes

BF16 = ml_dtypes.bfloat16


class Cfg:
    def __init__(self, N, E, G, P=8, D=64, TW=64, NCH=4, CS=256, SW=14):
        self.N, self.E, self.G, self.P, self.D = N, E, G, P, D
        self.TW = TW          # target window width (matmul M dim)
        self.NCH = NCH        # z-table chunks (int16 gather index range)
        self.CS = CS          # padded slots per (window, chunk) cell
        self.SW = SW          # windows per supertile
        self.NSH = N // P                     # nodes per core
        self.NJ = (self.NSH + 127) // 128     # node 128-groups per core
        self.NSHP = self.NJ * 128             # padded nodes per core
        self.NWIN = (self.NSH + TW - 1) // TW  # target windows per core
        assert self.NWIN % 2 == 0, "windows must pair into 128-partition PSUM"
        assert self.NWIN % SW == 0
        assert SW % 2 == 0
        self.NSUP = self.NWIN // SW
        self.QG = CS // 128                   # 128-slot groups per cell
        assert CS % 128 == 0
        # chunk c holds quarter c of every core's shard, core-major:
        # j-group quarters of NJ
        qs = [self.NJ // NCH + (1 if i < self.NJ % NCH else 0)
              for i in range(NCH)]
        self.QJ = qs                          # j-groups per quarter
        self.QSTART = np.concatenate([[0], np.cumsum(qs)]).astype(np.int64)
        self.QSH = [q * 128 for q in qs]      # rows per quarter per core
        self.CHSZ = [self.P * q for q in self.QSH]  # table rows per chunk
        assert max(self.CHSZ) <= 32767, "int16 gather index range"
        self.CHBASE = np.concatenate(
            [[0], np.cumsum(self.CHSZ)]).astype(np.int64)
        self.TROWS = self.P * self.NSHP       # z table rows (padded, global)
        assert self.TROWS == self.CHBASE[-1]
        self.HALVES = (G + 127) // 128        # 128-graph groups for pooling
        self.GP = min(G, 128)                 # graph partitions per half
        assert G % self.GP == 0
        # slots per (chunk, supertile) / per chunk / total
        self.SUP_SLOTS = SW * CS
        self.CH_SLOTS = self.NSUP * self.SUP_SLOTS
        self.TOT_SLOTS = NCH * self.CH_SLOTS
        # graphs per core for the pooled ReduceScatter shard
        assert G % P == 0
        self.GSH = G // P


CFG_FULL = Cfg(N=100000, E=1200000, G=256)


def host_prep(x, edge_index, batch, W, b, cfg):
    """Build per-core input maps. Index/layout transforms only."""
    c = cfg
    x = np.asarray(x, np.float32)
    W = np.asarray(W, np.float32)
    b = np.asarray(b, np.float32)
    row = np.asarray(edge_index[0], np.int64)
    col = np.asarray(edge_index[1], np.int64)
    batch = np.asarray(batch, np.int64)

    # global padded table position of a node: owner k, local l, quarter q
    # table row = CHBASE[q] + k*QSH[q] + (l - 128*QSTART[q])
    own_s = row // c.NSH
    loc_s = row - own_s * c.NSH
    jg = loc_s // 128
    qrt = np.searchsorted(c.QSTART, jg, "right") - 1
    qsh = np.asarray(c.QSH, np.int64)
    cidx64 = own_s * qsh[qrt] + (loc_s - 128 * c.QSTART[qrt])
    chunk = qrt
    cidx = cidx64.astype(np.int16)

    owner = col // c.NSH
    in_maps = []
    for k in range(c.P):
        m = owner == k
        c_l, ch, ci = (col[m] - k * c.NSH), chunk[m], cidx[m]
        w_l = c_l // c.TW
        t_l = (c_l % c.TW).astype(np.int64)

        # order edges by (chunk, window) cells; within-cell order arbitrary
        cell = ch * c.NWIN + w_l
        order = np.argsort(cell, kind="stable")
        cell_s, ci_s, t_s = cell[order], ci[order], t_l[order]
        counts = np.bincount(cell_s, minlength=c.NCH * c.NWIN)
        assert counts.max() <= c.CS, f"cell overflow {counts.max()} > {c.CS}"

        # slot arrays in (ch, sup, w_local, q*128+p) order
        gidx = np.zeros(c.TOT_SLOTS, np.int16)      # pad -> idx 0 (harmless)
        colv = np.full(c.TOT_SLOTS, -1, BF16)       # local target in window;
                                                    # pad -> -1 (one-hot 0)
        # position of cell (ch, w) in the slot stream:
        starts_cell = np.concatenate([[0], np.cumsum(counts)[:-1]])
        w_of_cell = np.arange(c.NCH * c.NWIN) % c.NWIN
        ch_of_cell = np.arange(c.NCH * c.NWIN) // c.NWIN
        base_of_cell = (ch_of_cell * c.CH_SLOTS
                        + (w_of_cell // c.SW) * c.SUP_SLOTS
                        + (w_of_cell % c.SW) * c.CS)
        pos_in_cell = np.arange(len(cell_s)) - starts_cell[cell_s]
        slot = base_of_cell[cell_s] + pos_in_cell
        gidx[slot] = ci_s
        colv[slot] = t_s.astype(BF16)

        # wrapped int16 index layout [NCH, NSUP, 128, SUP_SLOTS/16]:
        # stream pos i -> [i%16, i//16], replicated across 8 GPSIMD cores
        gidx_w = np.tile(
            gidx.reshape(c.NCH, c.NSUP, -1, 16).transpose(0, 1, 3, 2),
            (1, 1, 8, 1)).copy()
        # col values [NCH, NSUP, 128, SUP_SLOTS/128]: slot i -> [i%128, i//128]
        colv_w = (colv.reshape(c.NCH, c.NSUP, c.SUP_SLOTS // 128, 128)
                  .transpose(0, 1, 3, 2).copy())

        # CSR-style cumulative counts of sorted local cols (for degree)
        colk = np.sort(c_l)
        nodes = np.arange(c.NSHP)
        st = np.searchsorted(colk, nodes, "left").astype(np.float32)
        en = np.searchsorted(colk, nodes, "right").astype(np.float32)
        st_w = st.reshape(c.NJ, 128).T.copy()   # node j*128+p -> [p, j]
        en_w = en.reshape(c.NJ, 128).T.copy()

        # node features, transposed, padded
        xk = np.zeros((c.NSHP, c.D), np.float32)
        xk[: c.NSH] = x[k * c.NSH:(k + 1) * c.NSH]
        xT = xk.T.copy()

        # batch id per node (bf16 exact for < 256); pad nodes -> -1
        bk = np.full(c.NSHP, -1.0, BF16)
        bk[: c.NSH] = batch[k * c.NSH:(k + 1) * c.NSH].astype(BF16)
        bk_w = bk.reshape(c.NJ, 128).T.copy()

        b_rep = np.broadcast_to(b, (128, c.D)).copy()

        in_maps.append({
            "xT": xT,
            "Wm": W.copy(),
            "b_rep": b_rep,
            "starts": st_w,
            "ends": en_w,
            "gidx": gidx_w,
            "colv": colv_w,
            "batchv": bk_w,
        })
    return in_maps


def build_nc(cfg, scratch=32768):
    import sys
    if "/opt/trn_rl_repo" not in sys.path:
        sys.path.insert(0, "/opt/trn_rl_repo")
    from concourse import bass, mybir
    from concourse import bacc
    from concourse.tile import TileContext

    c = cfg
    f32, bf16, i16 = mybir.dt.float32, mybir.dt.bfloat16, mybir.dt.int16
    AF = mybir.ActivationFunctionType
    OP = mybir.AluOpType

    nc = bacc.Bacc(None, target_bir_lowering=False, num_swdge_queues=4,
                   dynamic_dma_scratch_size=scratch)
    xT_d = nc.declare_dram_parameter("xT", [c.D, c.NSHP], f32, isOutput=False)
    W_d = nc.declare_dram_parameter("Wm", [c.D, c.D], f32, isOutput=False)
    b_d = nc.declare_dram_parameter("b_rep", [128, c.D], f32, isOutput=False)
    st_d = nc.declare_dram_parameter("starts", [128, c.NJ], f32, isOutput=False)
    en_d = nc.declare_dram_parameter("ends", [128, c.NJ], f32, isOutput=False)
    gi_d = nc.declare_dram_parameter(
        "gidx", [c.NCH, c.NSUP, 128, c.SUP_SLOTS // 16], i16, isOutput=False)
    cv_d = nc.declare_dram_parameter(
        "colv", [c.NCH, c.NSUP, 128, c.SUP_SLOTS // 128], bf16, isOutput=False)
    bt_d = nc.declare_dram_parameter(
        "batchv", [128, c.NJ], bf16, isOutput=False)
    out_d = nc.declare_dram_parameter("out", [c.G, c.D], f32, isOutput=True)

    zk_dram = nc.dram_tensor("zk_dram", [c.NSHP, 128], bf16)
    z_ch = [
        nc.dram_tensor(f"z_ch{i}", [c.CHSZ[i], 128], bf16,
                       addr_space="Shared")
        for i in range(c.NCH)
    ]
    rs_in = nc.dram_tensor("rs_in", [c.G, c.D], f32)
    rs_out = nc.dram_tensor("rs_out", [c.GSH, c.D], f32)
    ag_out = nc.dram_tensor("ag_out", [c.G, c.D], f32, addr_space="Shared")
    groups = [list(range(c.P))]

    with TileContext(nc, num_cores=c.P) as tc:
        with (
            tc.tile_pool(name="const", bufs=1) as const_pool,
            tc.tile_pool(name="persist", bufs=1) as pp,
            tc.tile_pool(name="xs", bufs=2) as xs_pool,
            tc.tile_pool(name="zg", bufs=2) as zg_pool,
            tc.tile_pool(name="mt", bufs=2) as m_pool,
            tc.tile_pool(name="pb", bufs=2) as pb_pool,
            tc.tile_pool(name="psum", bufs=4, space="PSUM") as psum_pool,
            tc.tile_pool(name="psum1", bufs=1, space="PSUM") as psum1_pool,
        ):
            # ---- load constants ----
            W_sb = const_pool.tile([c.D, c.D], f32)
            b_sb = const_pool.tile([128, c.D], f32)
            st_sb = const_pool.tile([128, c.NJ], f32)
            en_sb = const_pool.tile([128, c.NJ], f32)
            nc.sync.dma_start(out=W_sb[:, :], in_=W_d[:, :])
            nc.sync.dma_start(out=b_sb[:, :], in_=b_d[:, :])
            nc.sync.dma_start(out=st_sb[:, :], in_=st_d[:, :])
            nc.sync.dma_start(out=en_sb[:, :], in_=en_d[:, :])
            bt_sb = const_pool.tile([128, c.NJ], bf16)
            nc.sync.dma_start(out=bt_sb[:, :], in_=bt_d[:, :])
            iota_i = const_pool.tile([128, c.G], i16)
            nc.gpsimd.iota(iota_i[:, :], pattern=[[1, c.G]],
                           channel_multiplier=0)
            iota_bf = const_pool.tile([128, c.G], bf16)
            nc.vector.tensor_copy(out=iota_bf[:, :], in_=iota_i[:, :])

            # ---- degree -> dinv ----
            deg = pp.tile([128, c.NJ], f32)
            dinv = pp.tile([128, c.NJ], f32)
            nc.vector.tensor_tensor(
                out=deg[:, :], in0=en_sb[:, :], in1=st_sb[:, :],
                op=OP.subtract)
            # dinv = 1/sqrt(deg + 1): self loop included analytically
            nc.vector.tensor_scalar_add(deg[:, :], deg[:, :], 1.0)
            nc.vector.reciprocal(out=deg[:, :], in_=deg[:, :])
            nc.scalar.activation(
                out=dinv[:, :], in_=deg[:, :], func=AF.Sqrt)

            # ---- h = x @ W (streamed xT), z = dinv * h (bf16) ----
            h_sb = pp.tile([128, c.NJ, c.D], f32, tag="slotA", name="h_sb")
            JB = 8  # j-groups per xT stream chunk / PSUM bank
            for j0 in range(0, c.NJ, JB):
                jn = min(JB, c.NJ - j0)
                xT_sb = xs_pool.tile([c.D, JB * 128], f32, tag="xs")
                nc.sync.dma_start(
                    out=xT_sb[:, 0:jn * 128],
                    in_=xT_d[:, j0 * 128:(j0 + jn) * 128])
                h_ps = psum_pool.tile([128, JB * c.D], f32, tag="agg",
                                      name="h_ps")
                for jj in range(jn):
                    nc.tensor.matmul(
                        h_ps[:, jj * c.D:(jj + 1) * c.D],
                        lhsT=xT_sb[:, jj * 128:(jj + 1) * 128],
                        rhs=W_sb[:, :], start=True, stop=True)
                nc.scalar.activation(
                    out=h_sb[:, j0:j0 + jn, :].rearrange("p j d -> p (j d)"),
                    in_=h_ps[:, 0:jn * c.D], func=AF.Copy)
            z_bf = pp.tile([128, c.NJ, c.D], bf16, tag="zbf", name="z_bf")
            nc.vector.tensor_tensor(
                out=z_bf[:, :, :], in0=h_sb[:, :, :],
                in1=dinv[:, :].unsqueeze(2).broadcast_to([128, c.NJ, c.D]),
                op=OP.mult)

            # ---- z shard -> DRAM; chunked AllGathers (pipelined) ----
            # pad half of each 256B row left unwritten: gathered into
            # zg[..., 64:128] but never read by the matmuls
            zk_view = zk_dram[:, :].rearrange("(j p) e -> p j e", p=128)
            nc.sync.dma_start(out=zk_view[:, :, 0:c.D], in_=z_bf[:, :, :])
            for i in range(c.NCH):
                nc.gpsimd.collective_compute(
                    "AllGather", OP.bypass, replica_groups=groups,
                    ins=[zk_dram[128 * c.QSTART[i]:128 * c.QSTART[i + 1], :]],
                    outs=[z_ch[i][:, :]])

            # ---- main loop: gather + one-hot matmul aggregation ----
            s_full = pp.tile([128, c.NJ, c.D], f32, tag="slotA",
                             name="s_full")
            # <=896 idx per gather call: SWDGE ring holds ~1023 16B descs
            GI = 896
            assert c.SUP_SLOTS % GI == 0
            for cch in range(c.NCH):
                for sup in range(c.NSUP):
                    cv_sb = m_pool.tile([128, c.SUP_SLOTS // 128], bf16,
                                        tag="cv", name="cv_sb")
                    nc.sync.dma_start(out=cv_sb[:, :], in_=cv_d[cch, sup, :, :])
                    m_sb = m_pool.tile(
                        [128, c.SW, c.QG, c.TW], bf16, tag="m")
                    nc.vector.tensor_tensor(
                        out=m_sb[:, :, :, :].rearrange(
                            "p w q t -> p (w q) t"),
                        in0=cv_sb[:, :].unsqueeze(2).broadcast_to(
                            [128, c.SUP_SLOTS // 128, c.TW]),
                        in1=iota_bf[:, 0:c.TW].unsqueeze(1).broadcast_to(
                            [128, c.SUP_SLOTS // 128, c.TW]),
                        op=OP.is_equal)
                    gix = m_pool.tile([128, c.SUP_SLOTS // 16], i16,
                                      tag="gix", name="gix")
                    nc.sync.dma_start(out=gix[:, :], in_=gi_d[cch, sup, :, :])
                    zg = zg_pool.tile(
                        [128, c.SW, c.QG, 128], bf16, tag="zg")
                    zflat = zg[:, :, :, :].rearrange("p w q e -> p (w q) e")
                    ng = c.SUP_SLOTS // GI
                    for h in range(ng):
                        nc.gpsimd.dma_gather(
                            zflat[:, h * (GI // 128):(h + 1) * (GI // 128), :],
                            z_ch[cch][:, :],
                            gix[:, h * (GI // 16):(h + 1) * (GI // 16)],
                            num_idxs=GI,
                            num_idxs_reg=GI,
                            elem_size=128,
                            queue_num=((cch * c.NSUP + sup) * ng + h) % 4,
                        )
                    for pair in range(c.SW // 2):
                        ps = psum_pool.tile([128, c.D], f32, tag="agg")
                        for half in range(2):
                            w = pair * 2 + half
                            for q in range(c.QG):
                                nc.tensor.matmul(
                                    ps[half * c.TW:(half + 1) * c.TW, :],
                                    lhsT=m_sb[:, w, q, :],
                                    rhs=zg[:, w, q, 0:c.D],
                                    start=(q == 0), stop=(q == c.QG - 1))
                        j = sup * (c.SW // 2) + pair
                        if cch == 0:
                            nc.scalar.activation(
                                out=s_full[:, j, :], in_=ps[:, :],
                                func=AF.Copy)
                        else:
                            nc.vector.tensor_tensor(
                                out=s_full[:, j, :], in0=ps[:, :],
                                in1=s_full[:, j, :], op=OP.add)

            # ---- post: relu(dinv*(S+z) + b), in place on s_full ----
            nc.vector.tensor_tensor(
                out=s_full[:, :, :], in0=s_full[:, :, :], in1=z_bf[:, :, :],
                op=OP.add)
            nc.vector.tensor_tensor(
                out=s_full[:, :, :], in0=s_full[:, :, :],
                in1=dinv[:, :].unsqueeze(2).broadcast_to([128, c.NJ, c.D]),
                op=OP.mult)
            nc.vector.tensor_tensor(
                out=s_full[:, :, :], in0=s_full[:, :, :],
                in1=b_sb[:, :].unsqueeze(1).broadcast_to([128, c.NJ, c.D]),
                op=OP.add)
            act = pp.tile([128, c.NJ, c.D], bf16, tag="zbf", name="act")
            nc.scalar.activation(
                out=act[:, :, :], in_=s_full[:, :, :], func=AF.Relu)

            # ---- pooling: one-hot matmul over node groups ----
            pool_ps = [
                psum1_pool.tile([c.GP, c.D], f32, tag=f"pool{h}",
                                name=f"pool_ps{h}")
                for h in range(c.HALVES)
            ]
            for j in range(c.NJ):
                pB_sb = pb_pool.tile([128, c.G], bf16, tag="pb")
                nc.vector.tensor_tensor(
                    out=pB_sb[:, :],
                    in0=bt_sb[:, j:j + 1].broadcast_to([128, c.G]),
                    in1=iota_bf[:, :],
                    op=OP.is_equal)
                for h in range(c.HALVES):
                    nc.tensor.matmul(
                        pool_ps[h][:, :],
                        lhsT=pB_sb[:, h * c.GP:(h + 1) * c.GP],
                        rhs=act[:, j, :],
                        start=(j == 0), stop=(j == c.NJ - 1))
            pool_sb = pp.tile([c.GP, c.HALVES, c.D], f32)
            for h in range(c.HALVES):
                nc.scalar.activation(
                    out=pool_sb[:, h, :], in_=pool_ps[h][:, :], func=AF.Copy)
            rs_view = rs_in[:, :].rearrange("(h p) d -> p h d", p=c.GP)
            nc.sync.dma_start(out=rs_view[:, :, :], in_=pool_sb[:, :, :])

            # ---- ReduceScatter + AllGather + output ----
            nc.gpsimd.collective_compute(
                "ReduceScatter", OP.add, replica_groups=groups,
                ins=[rs_in[:, :]], outs=[rs_out[:, :]])
            nc.gpsimd.collective_compute(
                "AllGather", OP.bypass, replica_groups=groups,
                ins=[rs_out[:, :]], outs=[ag_out[:, :]])
            nc.sync.dma_start(out=out_d[:, :], in_=ag_out[:, :])

    nc.finalize()
    return nc


_CACHE = {}
LAST_EXEC_NS = None
LAST_RESULT = None


def kernel(x, edge_index, batch, W, b):
    """Full inputs in, full [256, 64] output out; runs SPMD on 8 cores."""
    import os
    import sys
    if "/opt/trn_rl_repo" not in sys.path:
        sys.path.insert(0, "/opt/trn_rl_repo")
    from concourse.bass_utils import run_bass_kernel_spmd

    cfg = CFG_FULL
    in_maps = host_prep(x, edge_index, batch, W, b, cfg)
    if "nc" not in _CACHE:
        _CACHE["nc"] = build_nc(cfg)
    kw = {}
    tdir = os.environ.get("BASS_TRACE_DIR")
    if tdir:
        kw["tmpdir"] = tdir
    res = run_bass_kernel_spmd(_CACHE["nc"], in_maps, list(range(cfg.P)), **kw)
    global LAST_EXEC_NS, LAST_RESULT
    LAST_EXEC_NS = res.exec_time_ns
    LAST_RESULT = res
    return np.asarray(res.results[0]["out"], np.float32)


# revision 8
# speedup vs baseline: 1.8429x; 1.8429x over previous
"""GCN encoder (GCNConv + relu + global_add_pool) as a distributed Bass kernel
for 8 TRN2 NeuronCores.

Sharding strategy (edge-parallel per the hint):
  - Nodes sharded contiguously: core k owns nodes [k*NSH, (k+1)*NSH).
  - Edges partitioned by TARGET owner; each core aggregates its own targets.
  - Per-core z-table shard z_k = dinv_k * (x_k @ W) built on device, then
    AllGather -> full table in every core's DRAM.
  - Per-edge messages fetched with dma_gather (256B rows), aggregated per
    64-target window with one-hot matmuls on the TensorEngine (PSUM accum).
  - Self loops folded in analytically: agg[t] = dinv[t] * (S[t] + z[t]).
  - relu, then graph pooling via one-hot matmul, AllReduce of [G, 64].

Host-side prep does ONLY index/layout work (bucketing, sorting, one-hot
encodings of index structure, CSR-style cumulative counts). All value math
(degree arithmetic, rsqrt, x@W, scaling, aggregation, relu, pooling) runs on
device.
"""

import numpy as np
import ml_dtypes

BF16 = ml_dtypes.bfloat16


class Cfg:
    def __init__(self, N, E, G, P=8, D=64, TW=64, NCH=4, CS=256, SW=14):
        self.N, self.E, self.G, self.P, self.D = N, E, G, P, D
        self.TW = TW          # target window width (matmul M dim)
        self.NCH = NCH        # source chunks (int16 gather index range)
        self.CS = CS          # padded slots per (window, chunk) cell
        self.SW = SW          # windows per supertile
        self.NSH = N // P                     # nodes per core
        self.NJ = (self.NSH + 127) // 128     # node 128-groups per core
        self.NSHP = self.NJ * 128             # padded nodes per core
        self.NWIN = (self.NSH + TW - 1) // TW  # target windows per core
        assert self.NWIN % 2 == 0, "windows must pair into 128-partition PSUM"
        assert self.NWIN % SW == 0
        assert SW % 2 == 0
        self.NSUP = self.NWIN // SW
        self.QG = CS // 128                   # 128-slot groups per cell
        assert CS % 128 == 0
        self.TROWS = P * self.NSHP            # z table rows (padded, global)
        assert self.TROWS % NCH == 0
        self.CHSZ = self.TROWS // NCH         # table rows per chunk
        assert self.CHSZ <= 32767, "int16 gather index range"
        self.HALVES = (G + 127) // 128        # 128-graph groups for pooling
        self.GP = min(G, 128)                 # graph partitions per half
        assert G % self.GP == 0
        # slots per supertile / total
        self.SUP_SLOTS = SW * NCH * CS
        self.TOT_SLOTS = self.NSUP * self.SUP_SLOTS


CFG_FULL = Cfg(N=100000, E=1200000, G=256)


def host_prep(x, edge_index, batch, W, b, cfg):
    """Build per-core input maps. Index/layout transforms only."""
    c = cfg
    x = np.asarray(x, np.float32)
    W = np.asarray(W, np.float32)
    b = np.asarray(b, np.float32)
    row = np.asarray(edge_index[0], np.int64)
    col = np.asarray(edge_index[1], np.int64)
    batch = np.asarray(batch, np.int64)

    # global padded table row of a node
    trow = (row // c.NSH) * c.NSHP + (row % c.NSH)
    chunk = trow // c.CHSZ
    cidx = (trow % c.CHSZ).astype(np.int16)

    owner = col // c.NSH
    in_maps = []
    for k in range(c.P):
        m = owner == k
        r_t, c_l, ch, ci = trow[m], (col[m] - k * c.NSH), chunk[m], cidx[m]
        w_l = c_l // c.TW
        t_l = (c_l % c.TW).astype(np.int64)

        # order edges by (chunk, window) cells; within-cell order arbitrary
        cell = ch * c.NWIN + w_l
        order = np.argsort(cell, kind="stable")
        cell_s, ci_s, t_s = cell[order], ci[order], t_l[order]
        counts = np.bincount(cell_s, minlength=c.NCH * c.NWIN)
        assert counts.max() <= c.CS, f"cell overflow {counts.max()} > {c.CS}"

        # slot arrays in (sup, c, w_local, q*128+p) order
        gidx = np.zeros(c.TOT_SLOTS, np.int16)      # pad -> idx 0 (harmless)
        colv = np.full(c.TOT_SLOTS, -1, BF16)       # local target in window;
                                                    # pad -> -1 (one-hot row 0)
        # position of cell (ch, w) in the slot stream:
        #   sup = w // SW ; base = sup*SUP_SLOTS + ch*(SW*CS) + (w%SW)*CS
        starts_cell = np.concatenate([[0], np.cumsum(counts)[:-1]])
        w_of_cell = np.arange(c.NCH * c.NWIN) % c.NWIN
        ch_of_cell = np.arange(c.NCH * c.NWIN) // c.NWIN
        base_of_cell = ((w_of_cell // c.SW) * c.SUP_SLOTS
                        + ch_of_cell * (c.SW * c.CS)
                        + (w_of_cell % c.SW) * c.CS)
        pos_in_cell = np.arange(len(cell_s)) - starts_cell[cell_s]
        slot = base_of_cell[cell_s] + pos_in_cell
        gidx[slot] = ci_s
        colv[slot] = t_s.astype(BF16)

        # wrapped int16 index layout per supertile [NSUP, 128, SUP/16]:
        # stream pos i -> [i%16, i//16], replicated across the 8 GPSIMD cores
        gidx_w = np.tile(
            gidx.reshape(c.NSUP, -1, 16).transpose(0, 2, 1), (1, 8, 1)).copy()
        # col values [NSUP, 128, SUP/128]: slot i of sup -> [i%128, i//128]
        colv_w = (colv.reshape(c.NSUP, c.SUP_SLOTS // 128, 128)
                  .transpose(0, 2, 1).copy())

        # CSR-style cumulative counts of sorted local cols (for degree)
        colk = np.sort(c_l)
        nodes = np.arange(c.NSHP)
        st = np.searchsorted(colk, nodes, "left").astype(np.float32)
        en = np.searchsorted(colk, nodes, "right").astype(np.float32)
        st_w = st.reshape(c.NJ, 128).T.copy()   # node j*128+p -> [p, j]
        en_w = en.reshape(c.NJ, 128).T.copy()

        # node features, transposed, padded
        xk = np.zeros((c.NSHP, c.D), np.float32)
        xk[: c.NSH] = x[k * c.NSH:(k + 1) * c.NSH]
        xT = xk.T.copy()

        # batch id per node (bf16 exact for < 256); pad nodes -> -1
        bk = np.full(c.NSHP, -1.0, BF16)
        bk[: c.NSH] = batch[k * c.NSH:(k + 1) * c.NSH].astype(BF16)
        bk_w = bk.reshape(c.NJ, 128).T.copy()

        b_rep = np.broadcast_to(b, (128, c.D)).copy()

        in_maps.append({
            "xT": xT,
            "Wm": W.copy(),
            "b_rep": b_rep,
            "starts": st_w,
            "ends": en_w,
            "gidx": gidx_w,
            "colv": colv_w,
            "batchv": bk_w,
        })
    return in_maps


def build_nc(cfg, max_sup=None, skip_gather=False, skip_mm=False):
    import sys
    if "/opt/trn_rl_repo" not in sys.path:
        sys.path.insert(0, "/opt/trn_rl_repo")
    from concourse import bass, mybir
    from concourse import bacc
    from concourse.tile import TileContext

    c = cfg
    f32, bf16, i16 = mybir.dt.float32, mybir.dt.bfloat16, mybir.dt.int16
    AF = mybir.ActivationFunctionType
    OP = mybir.AluOpType

    nc = bacc.Bacc(None, target_bir_lowering=False, num_swdge_queues=4,
                   dynamic_dma_scratch_size=32768)
    xT_d = nc.declare_dram_parameter("xT", [c.D, c.NSHP], f32, isOutput=False)
    W_d = nc.declare_dram_parameter("Wm", [c.D, c.D], f32, isOutput=False)
    b_d = nc.declare_dram_parameter("b_rep", [128, c.D], f32, isOutput=False)
    st_d = nc.declare_dram_parameter("starts", [128, c.NJ], f32, isOutput=False)
    en_d = nc.declare_dram_parameter("ends", [128, c.NJ], f32, isOutput=False)
    gi_d = nc.declare_dram_parameter(
        "gidx", [c.NSUP, 128, c.SUP_SLOTS // 16], i16, isOutput=False)
    cv_d = nc.declare_dram_parameter(
        "colv", [c.NSUP, 128, c.SUP_SLOTS // 128], bf16, isOutput=False)
    bt_d = nc.declare_dram_parameter(
        "batchv", [128, c.NJ], bf16, isOutput=False)
    out_d = nc.declare_dram_parameter("out", [c.G, c.D], f32, isOutput=True)

    zk_dram = nc.dram_tensor("zk_dram", [c.NSHP, 128], bf16)
    z_full = nc.dram_tensor("z_full", [c.TROWS, 128], bf16,
                            addr_space="Shared")
    ar_in = nc.dram_tensor("ar_in", [c.G, c.D], f32)
    ar_out = nc.dram_tensor("ar_out", [c.G, c.D], f32, addr_space="Shared")
    groups = [list(range(c.P))]

    with TileContext(nc, num_cores=c.P) as tc:
        with (
            tc.tile_pool(name="const", bufs=1) as const_pool,
            tc.tile_pool(name="persist", bufs=1) as pp,
            tc.tile_pool(name="zg", bufs=2) as zg_pool,
            tc.tile_pool(name="mt", bufs=2) as m_pool,
            tc.tile_pool(name="psum", bufs=4, space="PSUM") as psum_pool,
            tc.tile_pool(name="psum1", bufs=1, space="PSUM") as psum1_pool,
        ):
            # ---- load constants / inputs ----
            # xT and poolB share one big slot (disjoint lifetimes)
            big_pool = const_pool  # alias for clarity; tag-based sharing
            xT = big_pool.tile([128, c.NSHP], f32, tag="bigslot", name="xT")
            W_sb = const_pool.tile([c.D, c.D], f32)
            b_sb = const_pool.tile([128, c.D], f32)
            st_sb = const_pool.tile([128, c.NJ], f32)
            en_sb = const_pool.tile([128, c.NJ], f32)
            nc.sync.dma_start(out=xT[0:c.D, :], in_=xT_d[:, :])
            nc.sync.dma_start(out=W_sb[:, :], in_=W_d[:, :])
            nc.sync.dma_start(out=b_sb[:, :], in_=b_d[:, :])
            nc.sync.dma_start(out=st_sb[:, :], in_=st_d[:, :])
            nc.sync.dma_start(out=en_sb[:, :], in_=en_d[:, :])
            bt_sb = const_pool.tile([128, c.NJ], bf16)
            nc.sync.dma_start(out=bt_sb[:, :], in_=bt_d[:, :])
            iota_i = const_pool.tile([128, c.G], i16)
            nc.gpsimd.iota(iota_i[:, :], pattern=[[1, c.G]],
                           channel_multiplier=0)
            iota_bf = const_pool.tile([128, c.G], bf16)
            nc.vector.tensor_copy(out=iota_bf[:, :], in_=iota_i[:, :])

            # ---- degree -> dinv ----
            deg = pp.tile([128, c.NJ], f32)
            dinv = pp.tile([128, c.NJ], f32)
            nc.vector.tensor_tensor(
                out=deg[:, :], in0=en_sb[:, :], in1=st_sb[:, :],
                op=OP.subtract)
            # dinv = 1/sqrt(deg + 1): self loop included analytically
            nc.vector.tensor_scalar_add(deg[:, :], deg[:, :], 1.0)
            nc.vector.reciprocal(out=deg[:, :], in_=deg[:, :])
            nc.scalar.activation(
                out=dinv[:, :], in_=deg[:, :], func=AF.Sqrt)

            # ---- h = x @ W  (per node 128-group), z = dinv * h (bf16) ----
            h_sb = pp.tile([128, c.NJ, c.D], f32, tag="slotA", name="h_sb")
            JB = 8  # j-groups per PSUM bank (8*64 f32 = 2KB bank)
            for j0 in range(0, c.NJ, JB):
                jn = min(JB, c.NJ - j0)
                h_ps = psum_pool.tile([128, JB * c.D], f32, tag="agg",
                                      name="h_ps")
                for jj in range(jn):
                    nc.tensor.matmul(
                        h_ps[:, jj * c.D:(jj + 1) * c.D],
                        lhsT=xT[0:c.D, (j0 + jj) * 128:(j0 + jj + 1) * 128],
                        rhs=W_sb[:, :], start=True, stop=True)
                nc.scalar.activation(
                    out=h_sb[:, j0:j0 + jn, :].rearrange("p j d -> p (j d)"),
                    in_=h_ps[:, 0:jn * c.D], func=AF.Copy)
            z_bf = pp.tile([128, c.NJ, c.D], bf16, tag="zbf", name="z_bf")
            nc.vector.tensor_tensor(
                out=z_bf[:, :, :], in0=h_sb[:, :, :],
                in1=dinv[:, :].unsqueeze(2).broadcast_to([128, c.NJ, c.D]),
                op=OP.mult)

            # ---- z table shard -> DRAM -> AllGather ----
            zk_view = zk_dram[:, :].rearrange("(j p) e -> p j e", p=128)
            nc.sync.dma_start(out=zk_view[:, :, 0:c.D], in_=z_bf[:, :, :])
            # fill the 256B-row pad half too (gathered but unused downstream)
            nc.sync.dma_start(out=zk_view[:, :, c.D:2 * c.D],
                              in_=z_bf[:, :, :])
            nc.gpsimd.collective_compute(
                "AllGather", OP.bypass, replica_groups=groups,
                ins=[zk_dram[:, :]], outs=[z_full[:, :]])

            # ---- pooling one-hot built on DVE into the slot xT used ----
            pB_sb = big_pool.tile([128, c.NJ, c.G], bf16, tag="bigslot",
                                  name="pB_sb")
            nc.vector.tensor_tensor(
                out=pB_sb[:, :, :],
                in0=bt_sb[:, :].unsqueeze(2).broadcast_to([128, c.NJ, c.G]),
                in1=iota_bf[:, :].unsqueeze(1).broadcast_to([128, c.NJ, c.G]),
                op=OP.is_equal)

            # ---- main loop: gather + one-hot matmul aggregation ----
            s_full = pp.tile([128, c.NJ, c.D], f32, tag="slotA",
                             name="s_full")
            nsup_run = c.NSUP if max_sup is None else max_sup
            if nsup_run < c.NSUP or skip_mm:
                nc.vector.memset(s_full[:, :, :], 0.0)
            blk16 = c.CS // 16
            for sup in range(nsup_run):
                cv_sb = m_pool.tile([128, c.SUP_SLOTS // 128], bf16,
                                    tag="cv", name="cv_sb")
                nc.sync.dma_start(out=cv_sb[:, :], in_=cv_d[sup, :, :])
                m_sb = m_pool.tile(
                    [128, c.NCH, c.SW, c.QG, c.TW], bf16, tag="m")
                nc.vector.tensor_tensor(
                    out=m_sb[:, :, :, :, :].rearrange(
                        "p ch w q t -> p (ch w q) t"),
                    in0=cv_sb[:, :].unsqueeze(2).broadcast_to(
                        [128, c.SUP_SLOTS // 128, c.TW]),
                    in1=iota_bf[:, 0:c.TW].unsqueeze(1).broadcast_to(
                        [128, c.SUP_SLOTS // 128, c.TW]),
                    op=OP.is_equal)
                gix = m_pool.tile([128, c.SUP_SLOTS // 16], i16,
                                  tag="gix", name="gix")
                nc.sync.dma_start(out=gix[:, :], in_=gi_d[sup, :, :])
                zg = zg_pool.tile(
                    [128, c.NCH, c.SW, c.QG, 128], bf16, tag="zg")
                # <=896 idx per gather: SWDGE ring holds ~1023 16B descs
                per_ch = c.SW * c.CS
                GI = min(896, per_ch)
                while per_ch % GI:
                    GI -= 128
                ng = per_ch // GI
                qg_per_g = GI // 128
                zflat = zg[:, :, :, :, :].rearrange("p ch w q e -> p (ch w q) e")
                for ch in range(c.NCH):
                    if skip_gather:
                        nc.vector.memset(
                            zg[:, ch, :, :, :].rearrange(
                                "p w q e -> p (w q) e"), 0.0)
                        continue
                    for h in range(ng):
                        base_qg = ch * (per_ch // 128) + h * qg_per_g
                        nc.gpsimd.dma_gather(
                            zflat[:, base_qg:base_qg + qg_per_g, :],
                            z_full[ch * c.CHSZ:(ch + 1) * c.CHSZ, :],
                            gix[:, (ch * per_ch + h * GI) // 16:
                                (ch * per_ch + (h + 1) * GI) // 16],
                            num_idxs=GI,
                            num_idxs_reg=GI,
                            elem_size=128,
                            queue_num=(ch * ng + h) % 4,
                        )
                for pair in range(c.SW // 2 if not skip_mm else 0):
                    ps = psum_pool.tile([128, c.D], f32, tag="agg")
                    for half in range(2):
                        w = pair * 2 + half
                        nmm = c.NCH * c.QG
                        i = 0
                        for ch in range(c.NCH):
                            for q in range(c.QG):
                                nc.tensor.matmul(
                                    ps[half * c.TW:(half + 1) * c.TW, :],
                                    lhsT=m_sb[:, ch, w, q, :],
                                    rhs=zg[:, ch, w, q, 0:c.D],
                                    start=(i == 0), stop=(i == nmm - 1))
                                i += 1
                    j = sup * (c.SW // 2) + pair
                    nc.scalar.activation(
                        out=s_full[:, j, :], in_=ps[:, :], func=AF.Copy)

            # ---- post: relu(dinv*(S+z) + b), in place on s_full ----
            nc.vector.tensor_tensor(
                out=s_full[:, :, :], in0=s_full[:, :, :], in1=z_bf[:, :, :],
                op=OP.add)
            nc.vector.tensor_tensor(
                out=s_full[:, :, :], in0=s_full[:, :, :],
                in1=dinv[:, :].unsqueeze(2).broadcast_to([128, c.NJ, c.D]),
                op=OP.mult)
            nc.vector.tensor_tensor(
                out=s_full[:, :, :], in0=s_full[:, :, :],
                in1=b_sb[:, :].unsqueeze(1).broadcast_to([128, c.NJ, c.D]),
                op=OP.add)
            act = pp.tile([128, c.NJ, c.D], bf16, tag="zbf", name="act")
            nc.scalar.activation(
                out=act[:, :, :], in_=s_full[:, :, :], func=AF.Relu)

            # ---- pooling: one-hot matmul over node groups ----
            pool_ps = [
                psum1_pool.tile([c.GP, c.D], f32, tag=f"pool{h}",
                                name=f"pool_ps{h}")
                for h in range(c.HALVES)
            ]
            for j in range(c.NJ):
                for h in range(c.HALVES):
                    nc.tensor.matmul(
                        pool_ps[h][:, :],
                        lhsT=pB_sb[:, j, h * c.GP:(h + 1) * c.GP],
                        rhs=act[:, j, :],
                        start=(j == 0), stop=(j == c.NJ - 1))
            pool_sb = pp.tile([c.GP, c.HALVES, c.D], f32)
            for h in range(c.HALVES):
                nc.scalar.activation(
                    out=pool_sb[:, h, :], in_=pool_ps[h][:, :], func=AF.Copy)
            ar_view = ar_in[:, :].rearrange("(h p) d -> p h d", p=c.GP)
            nc.sync.dma_start(out=ar_view[:, :, :], in_=pool_sb[:, :, :])

            # ---- AllReduce + output ----
            nc.gpsimd.collective_compute(
                "AllReduce", OP.add, replica_groups=groups,
                ins=[ar_in[:, :]], outs=[ar_out[:, :]])
            nc.sync.dma_start(out=out_d[:, :], in_=ar_out[:, :])

    nc.finalize()
    return nc


_CACHE = {}
LAST_EXEC_NS = None
LAST_RESULT = None


def kernel(x, edge_index, batch, W, b):
    """Full inputs in, full [256, 64] output out; runs SPMD on 8 cores."""
    import os
    import sys
    if "/opt/trn_rl_repo" not in sys.path:
        sys.path.insert(0, "/opt/trn_rl_repo")
    from concourse.bass_utils import run_bass_kernel_spmd

    cfg = CFG_FULL
    in_maps = host_prep(x, edge_index, batch, W, b, cfg)
    if "nc" not in _CACHE:
        _CACHE["nc"] = build_nc(cfg)
    kw = {}
    tdir = os.environ.get("BASS_TRACE_DIR")
    if tdir:
        kw["tmpdir"] = tdir
    res = run_bass_kernel_spmd(_CACHE["nc"], in_maps, list(range(cfg.P)), **kw)
    global LAST_EXEC_NS, LAST_RESULT
    LAST_EXEC_NS = res.exec_time_ns
    LAST_RESULT = res
    return np.asarray(res.results[0]["out"], np.float32)



# revision 9
# speedup vs baseline: 1.8649x; 1.0119x over previous
"""GCN encoder (GCNConv + relu + global_add_pool) as a distributed Bass kernel
for 8 TRN2 NeuronCores.

Sharding strategy (edge-parallel per the hint):
  - Nodes sharded contiguously: core k owns nodes [k*NSH, (k+1)*NSH).
  - Edges partitioned by TARGET owner; each core aggregates its own targets.
  - Per-core z-table shard z_k = dinv_k * (x_k @ W) built on device, then
    AllGather -> full table in every core's DRAM.
  - Per-edge messages fetched with dma_gather (256B rows), aggregated per
    64-target window with one-hot matmuls on the TensorEngine (PSUM accum).
  - Self loops folded in analytically: agg[t] = dinv[t] * (S[t] + z[t]).
  - relu, then graph pooling via one-hot matmul, AllReduce of [G, 64].

Host-side prep does ONLY index/layout work (bucketing, sorting, one-hot
encodings of index structure, CSR-style cumulative counts). All value math
(degree arithmetic, rsqrt, x@W, scaling, aggregation, relu, pooling) runs on
device.
"""

import numpy as np
import ml_dtypes

BF16 = ml_dtypes.bfloat16


class Cfg:
    def __init__(self, N, E, G, P=8, D=64, TW=64, NCH=4, CS=256, SW=14):
        self.N, self.E, self.G, self.P, self.D = N, E, G, P, D
        self.TW = TW          # target window width (matmul M dim)
        self.NCH = NCH        # source chunks (int16 gather index range)
        self.CS = CS          # padded slots per (window, chunk) cell
        self.SW = SW          # windows per supertile
        self.NSH = N // P                     # nodes per core
        self.NJ = (self.NSH + 127) // 128     # node 128-groups per core
        self.NSHP = self.NJ * 128             # padded nodes per core
        self.NWIN = (self.NSH + TW - 1) // TW  # target windows per core
        assert self.NWIN % 2 == 0, "windows must pair into 128-partition PSUM"
        assert self.NWIN % SW == 0
        assert SW % 2 == 0
        self.NSUP = self.NWIN // SW
        self.QG = CS // 128                   # 128-slot groups per cell
        assert CS % 128 == 0
        self.TROWS = P * self.NSHP            # z table rows (padded, global)
        assert self.TROWS % NCH == 0
        self.CHSZ = self.TROWS // NCH         # table rows per chunk
        assert self.CHSZ <= 32767, "int16 gather index range"
        self.HALVES = (G + 127) // 128        # 128-graph groups for pooling
        self.GP = min(G, 128)                 # graph partitions per half
        assert G % self.GP == 0
        # slots per supertile / total
        self.SUP_SLOTS = SW * NCH * CS
        self.TOT_SLOTS = self.NSUP * self.SUP_SLOTS


CFG_FULL = Cfg(N=100000, E=1200000, G=256)


def host_prep(x, edge_index, batch, W, b, cfg):
    """Build per-core input maps. Index/layout transforms only."""
    c = cfg
    x = np.asarray(x, np.float32)
    W = np.asarray(W, np.float32)
    b = np.asarray(b, np.float32)
    row = np.asarray(edge_index[0], np.int64)
    col = np.asarray(edge_index[1], np.int64)
    batch = np.asarray(batch, np.int64)

    # global padded table row of a node
    trow = (row // c.NSH) * c.NSHP + (row % c.NSH)
    chunk = trow // c.CHSZ
    cidx = (trow % c.CHSZ).astype(np.int16)

    owner = col // c.NSH
    in_maps = []
    for k in range(c.P):
        m = owner == k
        r_t, c_l, ch, ci = trow[m], (col[m] - k * c.NSH), chunk[m], cidx[m]
        w_l = c_l // c.TW
        t_l = (c_l % c.TW).astype(np.int64)

        # order edges by (chunk, window) cells; within-cell order arbitrary
        cell = ch * c.NWIN + w_l
        order = np.argsort(cell, kind="stable")
        cell_s, ci_s, t_s = cell[order], ci[order], t_l[order]
        counts = np.bincount(cell_s, minlength=c.NCH * c.NWIN)
        assert counts.max() <= c.CS, f"cell overflow {counts.max()} > {c.CS}"

        # slot arrays in (sup, c, w_local, q*128+p) order
        gidx = np.zeros(c.TOT_SLOTS, np.int16)      # pad -> idx 0 (harmless)
        colv = np.full(c.TOT_SLOTS, -1, BF16)       # local target in window;
                                                    # pad -> -1 (one-hot row 0)
        # position of cell (ch, w) in the slot stream:
        #   sup = w // SW ; base = sup*SUP_SLOTS + ch*(SW*CS) + (w%SW)*CS
        starts_cell = np.concatenate([[0], np.cumsum(counts)[:-1]])
        w_of_cell = np.arange(c.NCH * c.NWIN) % c.NWIN
        ch_of_cell = np.arange(c.NCH * c.NWIN) // c.NWIN
        base_of_cell = ((w_of_cell // c.SW) * c.SUP_SLOTS
                        + ch_of_cell * (c.SW * c.CS)
                        + (w_of_cell % c.SW) * c.CS)
        pos_in_cell = np.arange(len(cell_s)) - starts_cell[cell_s]
        slot = base_of_cell[cell_s] + pos_in_cell
        gidx[slot] = ci_s
        colv[slot] = t_s.astype(BF16)

        # wrapped int16 index layout per supertile [NSUP, 128, SUP/16]:
        # stream pos i -> [i%16, i//16], replicated across the 8 GPSIMD cores
        gidx_w = np.tile(
            gidx.reshape(c.NSUP, -1, 16).transpose(0, 2, 1), (1, 8, 1)).copy()
        # col values [NSUP, 128, SUP/128]: slot i of sup -> [i%128, i//128]
        colv_w = (colv.reshape(c.NSUP, c.SUP_SLOTS // 128, 128)
                  .transpose(0, 2, 1).copy())

        # CSR-style cumulative counts of sorted local cols (for degree)
        colk = np.sort(c_l)
        nodes = np.arange(c.NSHP)
        st = np.searchsorted(colk, nodes, "left").astype(np.float32)
        en = np.searchsorted(colk, nodes, "right").astype(np.float32)
        st_w = st.reshape(c.NJ, 128).T.copy()   # node j*128+p -> [p, j]
        en_w = en.reshape(c.NJ, 128).T.copy()

        # node features, transposed, padded
        xk = np.zeros((c.NSHP, c.D), np.float32)
        xk[: c.NSH] = x[k * c.NSH:(k + 1) * c.NSH]
        xT = xk.T.copy()

        # batch id per node (bf16 exact for < 256); pad nodes -> -1
        bk = np.full(c.NSHP, -1.0, BF16)
        bk[: c.NSH] = batch[k * c.NSH:(k + 1) * c.NSH].astype(BF16)
        bk_w = bk.reshape(c.NJ, 128).T.copy()

        b_rep = np.broadcast_to(b, (128, c.D)).copy()

        in_maps.append({
            "xT": xT,
            "Wm": W.copy(),
            "b_rep": b_rep,
            "starts": st_w,
            "ends": en_w,
            "gidx": gidx_w,
            "colv": colv_w,
            "batchv": bk_w,
        })
    return in_maps


def build_nc(cfg, max_sup=None, skip_gather=False, skip_mm=False):
    import sys
    if "/opt/trn_rl_repo" not in sys.path:
        sys.path.insert(0, "/opt/trn_rl_repo")
    from concourse import bass, mybir
    from concourse import bacc
    from concourse.tile import TileContext

    c = cfg
    f32, bf16, i16 = mybir.dt.float32, mybir.dt.bfloat16, mybir.dt.int16
    AF = mybir.ActivationFunctionType
    OP = mybir.AluOpType

    nc = bacc.Bacc(None, target_bir_lowering=False, num_swdge_queues=4,
                   dynamic_dma_scratch_size=32768)
    xT_d = nc.declare_dram_parameter("xT", [c.D, c.NSHP], f32, isOutput=False)
    W_d = nc.declare_dram_parameter("Wm", [c.D, c.D], f32, isOutput=False)
    b_d = nc.declare_dram_parameter("b_rep", [128, c.D], f32, isOutput=False)
    st_d = nc.declare_dram_parameter("starts", [128, c.NJ], f32, isOutput=False)
    en_d = nc.declare_dram_parameter("ends", [128, c.NJ], f32, isOutput=False)
    gi_d = nc.declare_dram_parameter(
        "gidx", [c.NSUP, 128, c.SUP_SLOTS // 16], i16, isOutput=False)
    cv_d = nc.declare_dram_parameter(
        "colv", [c.NSUP, 128, c.SUP_SLOTS // 128], bf16, isOutput=False)
    bt_d = nc.declare_dram_parameter(
        "batchv", [128, c.NJ], bf16, isOutput=False)
    out_d = nc.declare_dram_parameter("out", [c.G, c.D], f32, isOutput=True)

    zk_dram = nc.dram_tensor("zk_dram", [c.NSHP, 128], bf16)
    z_full = nc.dram_tensor("z_full", [c.TROWS, 128], bf16,
                            addr_space="Shared")
    rs_in = nc.dram_tensor("rs_in", [c.G, c.D], f32)
    rs_out = nc.dram_tensor("rs_out", [c.G // c.P, c.D], f32)
    ag_out = nc.dram_tensor("ag_out", [c.G, c.D], f32, addr_space="Shared")
    groups = [list(range(c.P))]

    with TileContext(nc, num_cores=c.P) as tc:
        with (
            tc.tile_pool(name="const", bufs=1) as const_pool,
            tc.tile_pool(name="persist", bufs=1) as pp,
            tc.tile_pool(name="zg", bufs=2) as zg_pool,
            tc.tile_pool(name="mt", bufs=2) as m_pool,
            tc.tile_pool(name="psum", bufs=4, space="PSUM") as psum_pool,
            tc.tile_pool(name="psum1", bufs=1, space="PSUM") as psum1_pool,
        ):
            # ---- load constants / inputs ----
            # xT and poolB share one big slot (disjoint lifetimes)
            big_pool = const_pool  # alias for clarity; tag-based sharing
            xT = big_pool.tile([128, c.NSHP], f32, tag="bigslot", name="xT")
            W_sb = const_pool.tile([c.D, c.D], f32)
            b_sb = const_pool.tile([128, c.D], f32)
            st_sb = const_pool.tile([128, c.NJ], f32)
            en_sb = const_pool.tile([128, c.NJ], f32)
            nc.sync.dma_start(out=xT[0:c.D, :], in_=xT_d[:, :])
            nc.sync.dma_start(out=W_sb[:, :], in_=W_d[:, :])
            nc.sync.dma_start(out=b_sb[:, :], in_=b_d[:, :])
            nc.sync.dma_start(out=st_sb[:, :], in_=st_d[:, :])
            nc.sync.dma_start(out=en_sb[:, :], in_=en_d[:, :])
            bt_sb = const_pool.tile([128, c.NJ], bf16)
            nc.sync.dma_start(out=bt_sb[:, :], in_=bt_d[:, :])
            iota_i = const_pool.tile([128, c.G], i16)
            nc.gpsimd.iota(iota_i[:, :], pattern=[[1, c.G]],
                           channel_multiplier=0)
            iota_bf = const_pool.tile([128, c.G], bf16)
            nc.vector.tensor_copy(out=iota_bf[:, :], in_=iota_i[:, :])

            # ---- degree -> dinv ----
            deg = pp.tile([128, c.NJ], f32)
            dinv = pp.tile([128, c.NJ], f32)
            nc.vector.tensor_tensor(
                out=deg[:, :], in0=en_sb[:, :], in1=st_sb[:, :],
                op=OP.subtract)
            # dinv = 1/sqrt(deg + 1): self loop included analytically
            nc.vector.tensor_scalar_add(deg[:, :], deg[:, :], 1.0)
            nc.vector.reciprocal(out=deg[:, :], in_=deg[:, :])
            nc.scalar.activation(
                out=dinv[:, :], in_=deg[:, :], func=AF.Sqrt)

            # ---- h = x @ W  (per node 128-group), z = dinv * h (bf16) ----
            h_sb = pp.tile([128, c.NJ, c.D], f32, tag="slotA", name="h_sb")
            JB = 8  # j-groups per PSUM bank (8*64 f32 = 2KB bank)
            for j0 in range(0, c.NJ, JB):
                jn = min(JB, c.NJ - j0)
                h_ps = psum_pool.tile([128, JB * c.D], f32, tag="agg",
                                      name="h_ps")
                for jj in range(jn):
                    nc.tensor.matmul(
                        h_ps[:, jj * c.D:(jj + 1) * c.D],
                        lhsT=xT[0:c.D, (j0 + jj) * 128:(j0 + jj + 1) * 128],
                        rhs=W_sb[:, :], start=True, stop=True)
                nc.scalar.activation(
                    out=h_sb[:, j0:j0 + jn, :].rearrange("p j d -> p (j d)"),
                    in_=h_ps[:, 0:jn * c.D], func=AF.Copy)
            z_bf = pp.tile([128, c.NJ, c.D], bf16, tag="zbf", name="z_bf")
            nc.vector.tensor_tensor(
                out=z_bf[:, :, :], in0=h_sb[:, :, :],
                in1=dinv[:, :].unsqueeze(2).broadcast_to([128, c.NJ, c.D]),
                op=OP.mult)

            # ---- z table shard -> DRAM -> AllGather ----
            zk_view = zk_dram[:, :].rearrange("(j p) e -> p j e", p=128)
            nc.sync.dma_start(out=zk_view[:, :, 0:c.D], in_=z_bf[:, :, :])
            # fill the 256B-row pad half too (gathered but unused downstream)
            nc.sync.dma_start(out=zk_view[:, :, c.D:2 * c.D],
                              in_=z_bf[:, :, :])
            nc.gpsimd.collective_compute(
                "AllGather", OP.bypass, replica_groups=groups,
                ins=[zk_dram[:, :]], outs=[z_full[:, :]])

            # ---- pooling one-hot built on DVE into the slot xT used ----
            pB_sb = big_pool.tile([128, c.NJ, c.G], bf16, tag="bigslot",
                                  name="pB_sb")
            nc.vector.tensor_tensor(
                out=pB_sb[:, :, :],
                in0=bt_sb[:, :].unsqueeze(2).broadcast_to([128, c.NJ, c.G]),
                in1=iota_bf[:, :].unsqueeze(1).broadcast_to([128, c.NJ, c.G]),
                op=OP.is_equal)

            # ---- main loop: gather + one-hot matmul aggregation ----
            s_full = pp.tile([128, c.NJ, c.D], f32, tag="slotA",
                             name="s_full")
            nsup_run = c.NSUP if max_sup is None else max_sup
            if nsup_run < c.NSUP or skip_mm:
                nc.vector.memset(s_full[:, :, :], 0.0)
            blk16 = c.CS // 16
            for sup in range(nsup_run):
                cv_sb = m_pool.tile([128, c.SUP_SLOTS // 128], bf16,
                                    tag="cv", name="cv_sb")
                nc.sync.dma_start(out=cv_sb[:, :], in_=cv_d[sup, :, :])
                m_sb = m_pool.tile(
                    [128, c.NCH, c.SW, c.QG, c.TW], bf16, tag="m")
                nc.vector.tensor_tensor(
                    out=m_sb[:, :, :, :, :].rearrange(
                        "p ch w q t -> p (ch w q) t"),
                    in0=cv_sb[:, :].unsqueeze(2).broadcast_to(
                        [128, c.SUP_SLOTS // 128, c.TW]),
                    in1=iota_bf[:, 0:c.TW].unsqueeze(1).broadcast_to(
                        [128, c.SUP_SLOTS // 128, c.TW]),
                    op=OP.is_equal)
                gix = m_pool.tile([128, c.SUP_SLOTS // 16], i16,
                                  tag="gix", name="gix")
                nc.sync.dma_start(out=gix[:, :], in_=gi_d[sup, :, :])
                zg = zg_pool.tile(
                    [128, c.NCH, c.SW, c.QG, 128], bf16, tag="zg")
                # <=896 idx per gather: SWDGE ring holds ~1023 16B descs
                per_ch = c.SW * c.CS
                GI = min(896, per_ch)
                while per_ch % GI:
                    GI -= 128
                ng = per_ch // GI
                qg_per_g = GI // 128
                zflat = zg[:, :, :, :, :].rearrange("p ch w q e -> p (ch w q) e")
                for ch in range(c.NCH):
                    if skip_gather:
                        nc.vector.memset(
                            zg[:, ch, :, :, :].rearrange(
                                "p w q e -> p (w q) e"), 0.0)
                        continue
                    for h in range(ng):
                        base_qg = ch * (per_ch // 128) + h * qg_per_g
                        nc.gpsimd.dma_gather(
                            zflat[:, base_qg:base_qg + qg_per_g, :],
                            z_full[ch * c.CHSZ:(ch + 1) * c.CHSZ, :],
                            gix[:, (ch * per_ch + h * GI) // 16:
                                (ch * per_ch + (h + 1) * GI) // 16],
                            num_idxs=GI,
                            num_idxs_reg=GI,
                            elem_size=128,
                            queue_num=(ch * ng + h) % 4,
                        )
                for pair in range(c.SW // 2 if not skip_mm else 0):
                    ps = psum_pool.tile([128, c.D], f32, tag="agg")
                    for half in range(2):
                        w = pair * 2 + half
                        nmm = c.NCH * c.QG
                        i = 0
                        for ch in range(c.NCH):
                            for q in range(c.QG):
                                nc.tensor.matmul(
                                    ps[half * c.TW:(half + 1) * c.TW, :],
                                    lhsT=m_sb[:, ch, w, q, :],
                                    rhs=zg[:, ch, w, q, 0:c.D],
                                    start=(i == 0), stop=(i == nmm - 1))
                                i += 1
                    j = sup * (c.SW // 2) + pair
                    nc.scalar.activation(
                        out=s_full[:, j, :], in_=ps[:, :], func=AF.Copy)

            # ---- post: relu(dinv*(S+z) + b), in place on s_full ----
            nc.vector.tensor_tensor(
                out=s_full[:, :, :], in0=s_full[:, :, :], in1=z_bf[:, :, :],
                op=OP.add)
            nc.vector.tensor_tensor(
                out=s_full[:, :, :], in0=s_full[:, :, :],
                in1=dinv[:, :].unsqueeze(2).broadcast_to([128, c.NJ, c.D]),
                op=OP.mult)
            nc.vector.tensor_tensor(
                out=s_full[:, :, :], in0=s_full[:, :, :],
                in1=b_sb[:, :].unsqueeze(1).broadcast_to([128, c.NJ, c.D]),
                op=OP.add)
            act = pp.tile([128, c.NJ, c.D], bf16, tag="zbf", name="act")
            nc.scalar.activation(
                out=act[:, :, :], in_=s_full[:, :, :], func=AF.Relu)

            # ---- pooling: one-hot matmul over node groups ----
            pool_ps = [
                psum1_pool.tile([c.GP, c.D], f32, tag=f"pool{h}",
                                name=f"pool_ps{h}")
                for h in range(c.HALVES)
            ]
            for j in range(c.NJ):
                for h in range(c.HALVES):
                    nc.tensor.matmul(
                        pool_ps[h][:, :],
                        lhsT=pB_sb[:, j, h * c.GP:(h + 1) * c.GP],
                        rhs=act[:, j, :],
                        start=(j == 0), stop=(j == c.NJ - 1))
            pool_sb = pp.tile([c.GP, c.HALVES, c.D], f32)
            for h in range(c.HALVES):
                nc.scalar.activation(
                    out=pool_sb[:, h, :], in_=pool_ps[h][:, :], func=AF.Copy)
            rs_view = rs_in[:, :].rearrange("(h p) d -> p h d", p=c.GP)
            nc.sync.dma_start(out=rs_view[:, :, :], in_=pool_sb[:, :, :])

            # ---- ReduceScatter + AllGather + output ----
            nc.gpsimd.collective_compute(
                "ReduceScatter", OP.add, replica_groups=groups,
                ins=[rs_in[:, :]], outs=[rs_out[:, :]])
            nc.gpsimd.collective_compute(
                "AllGather", OP.bypass, replica_groups=groups,
                ins=[rs_out[:, :]], outs=[ag_out[:, :]])
            nc.sync.dma_start(out=out_d[:, :], in_=ag_out[:, :])

    nc.finalize()
    return nc


_CACHE = {}
LAST_EXEC_NS = None
LAST_RESULT = None


def kernel(x, edge_index, batch, W, b):
    """Full inputs in, full [256, 64] output out; runs SPMD on 8 cores."""
    import os
    import sys
    if "/opt/trn_rl_repo" not in sys.path:
        sys.path.insert(0, "/opt/trn_rl_repo")
    from concourse.bass_utils import run_bass_kernel_spmd

    cfg = CFG_FULL
    in_maps = host_prep(x, edge_index, batch, W, b, cfg)
    if "nc" not in _CACHE:
        _CACHE["nc"] = build_nc(cfg)
    kw = {}
    tdir = os.environ.get("BASS_TRACE_DIR")
    if tdir:
        kw["tmpdir"] = tdir
    res = run_bass_kernel_spmd(_CACHE["nc"], in_maps, list(range(cfg.P)), **kw)
    global LAST_EXEC_NS, LAST_RESULT
    LAST_EXEC_NS = res.exec_time_ns
    LAST_RESULT = res
    return np.asarray(res.results[0]["out"], np.float32)

